# revision 37
# baseline (speedup 1.0000x reference)
"""Trainium2 Bass kernel for nn_NUFFTLayerMultiChannelInitMixed.

Math: the reference's spread->FFT->filter->IFFT->energy pipeline is an exact
bilinear form in the analytic spectrum of the periodized Gaussians.  With the
M-aliased images dropped (their weight is exp(-tau*(k-M)^2) ~ 3e-5) the
energy reduces to a truncated cosine series in the K lowest modes:

  e_i[n] = sum_{k<K} g_ik * ( cs_k cos(k x_n) + ss_k sin(k x_n) ) - self_i
  cs_k   = sum_n cos(k x_n),   ss_k = sum_n sin(k x_n)
  g_ik   = pref * w_k * deconv^2_k * mult_ik * p_k^2   (host precomputed)

K=32 keeps rel err ~4e-4 (gate is 2e-2).  Each core handles BPC=2 batches;
partition p = 64*nhalf + 32*batch + k packs BOTH 512-point halves of BOTH
batches into the 128 partitions, so every elementwise stage is a single
[128, 512] instruction:

  phases   u = kst8^T @ t3p on PE (fp16 2-term split of t: phase err ~1e-4 rad)
  range    v = u + MAGIC; negr = (v - MAGIC) - u   (exact fp32 rounding trick)
  trig     h = sin(pi*negr); smat = sin(2*pi*u) via Sin ACT (accum -> ss)
           mneg = -2h^2 = cos - 1 (accum -> cs_half-512); cmat = mneg + 1
  energy   e = a^T cos + b^T sin, four block-masked fp16 stationaries
           (per n-half), self-energy and the +1024 fold baked host-side

No transposes, no collectives; output written [b, c, n] and transposed on
the host.
"""

import numpy as np

try:
    import concourse.bass as bass
except ImportError:
    import sys
    sys.path.insert(0, "/opt/trn_rl_repo")
    import concourse.bass as bass

import concourse.bacc as bacc
import concourse.mybir as mybir
from concourse import tile
from concourse.bass_utils import run_bass_kernel_spmd

F32 = mybir.dt.float32
F16 = mybir.dt.float16
AF = mybir.ActivationFunctionType
ALU = mybir.AluOpType

M = 2001
L = 2.0 * np.pi
TAU = 12.0 * (L / (2.0 * np.pi * M)) ** 2
K = 32                  # modes kept per batch
B_FULL, N = 16, 1024
NH = 512                # points per n-half
NCORES = 8
BPC = B_FULL // NCORES
MAGIC = 12582912.0      # 1.5 * 2^23: fl(u + MAGIC) - MAGIC = round-to-nearest(u)
PI = float(np.pi)


def _host_tables(shift0, shift1, amp0, amp1):
    """fp64 k-space tables for the packed layout p = 64*nh + 32*b + k.

    Returns kst12 [12,128] fp16 (phase stationary, rows r = 6*nh + 3*b + i)
    and gHHa [128,16] f32 = [gM_h0 | gM_h1 | HHa_h0 | HHa_h1]."""
    k = np.arange(K, dtype=np.float64)
    p = np.exp(-TAU * k * k)
    Cc = (M / L) * np.sqrt(4.0 * np.pi * TAU)
    deconv2 = (np.pi / TAU) * np.exp(2.0 * TAU * k * k)
    mult1 = float(amp0) * 4.0 * np.pi / (k * k + (1.0 * float(shift0)) ** 2)
    mult2 = float(amp1) * 4.0 * np.pi / (k * k + (0.5 * float(shift1)) ** 2)
    w = np.full(K, 2.0)
    w[0] = 1.0
    scale = 1.0 / ((2.0 * np.pi * M / L) * (2.0 * np.pi))
    pref = scale * Cc * Cc / M
    g = np.stack([pref * w * deconv2 * mult1 * p * p,
                  pref * w * deconv2 * mult2 * p * p], axis=1)   # [K, 2]
    self2 = g.sum(axis=0)

    pp = np.arange(128)
    nh_p, b_p, k_p = pp // 64, (pp // 32) % 2, pp % 32
    gM = np.zeros((2, 128, 4))
    HHa = np.zeros((2, 128, 4))
    for nh in range(2):
        rows = np.nonzero(nh_p == nh)[0]
        for r in rows:
            for c in range(2):
                col = 2 * b_p[r] + c
                gM[nh, r, col] = g[k_p[r], c]
                # cs arrives as csneg = cs - 1024: fold 1024*g into the bias
                HHa[nh, r, col] = 1024.0 * g[k_p[r], c] - (
                    self2[c] if k_p[r] == 0 else 0.0)
    gHHa = np.concatenate([gM[0], gM[1], HHa[0], HHa[1]],
                          axis=1).astype(np.float32)             # [128, 16]

    kst12 = np.zeros((8, 128), np.float32)
    for r in range(8):
        r_nh, r_b = r // 4, (r // 2) % 2
        sel = (nh_p == r_nh) & (b_p == r_b)
        kst12[r, sel] = k_p[sel]

    # fold matrix: cs2[p'] = sum_p Fold[p, p'] csh[p] adds the two n-halves
    fold = (pp[:, None] % 64 == pp[None, :] % 64).astype(np.float16)
    return kst12.astype(np.float16), gHHa, fold


def _split3(t):
    """t fp64 -> three fp16 arrays with t0+t1+t2 == t to ~2^-25."""
    t0 = t.astype(np.float16)
    r = t - t0.astype(np.float64)
    t1 = r.astype(np.float16)
    r = r - t1.astype(np.float64)
    t2 = r.astype(np.float16)
    return t0, t1, t2


def _build_program(debug=False):
    nc = bacc.Bacc(None, target_bir_lowering=False, debug=debug)
    # kt3 = [kst12 | t3p] packed fp16; gHHa = [gM_h0|gM_h1|HHa_h0|HHa_h1] f32
    kt3_in = nc.declare_dram_parameter("kt3", [8, 128 + NH], F16, isOutput=False)
    gHH_in = nc.declare_dram_parameter("gHH", [128, 16], F32, isOutput=False)
    fold_in = nc.declare_dram_parameter("fold", [128, 128], F16, isOutput=False)
    out_t = nc.declare_dram_parameter("out", [BPC, 2, N], F32, isOutput=True)

    with tile.TileContext(nc) as tc:
        import contextlib
        with contextlib.ExitStack() as ctx:
            pc = ctx.enter_context(tc.tile_pool(name="const", bufs=1))
            wp = ctx.enter_context(tc.tile_pool(name="work", bufs=1))
            ps_u = ctx.enter_context(tc.tile_pool(name="psu", bufs=1, space="PSUM"))
            ps_e = ctx.enter_context(tc.tile_pool(name="pse", bufs=1, space="PSUM"))

            kt3 = pc.tile([8, 128 + NH], F16, tag="kt3")
            nc.sync.dma_start(kt3[:], kt3_in[:])
            gHH = pc.tile([128, 16], F32, tag="gHH")
            nc.scalar.dma_start(gHH[:], gHH_in[:])
            fold = pc.tile([128, 128], F16, tag="fold")
            nc.sync.dma_start(fold[:], fold_in[:])
            kst = kt3[:, 0:128]
            t3p = kt3[:, 128 : 128 + NH]

            # phases u[p, n'] = k(p) * t_{b(p)}[512*nh(p) + n']
            u = ps_u.tile([128, NH], F32, tag="u")
            nc.tensor.matmul(u[:], kst, t3p, start=True, stop=True)

            # negr = round(u) - u (exact); sin(-2*pi*negr) = sin(2*pi*u)
            v = wp.tile([128, NH], F32, tag="v")
            nc.vector.tensor_scalar(v[:], u[:], MAGIC, None, ALU.add)
            negr = wp.tile([128, NH], F32, tag="negr")
            nc.vector.scalar_tensor_tensor(negr[:], v[:], MAGIC, u[:],
                                           ALU.subtract, ALU.subtract)

            h = wp.tile([128, NH], F16, tag="h")
            nc.scalar.activation(h[:], negr[:], AF.Sin, scale=PI)
            sscn = wp.tile([128, 1], F32, tag="sscn")
            smat = wp.tile([128, NH], F16, tag="smat")
            nc.scalar.activation(smat[:], negr[:], AF.Sin, scale=-2.0 * PI,
                                 accum_out=sscn[:])

            # mneg = -2 sin^2(pi r) = cos - 1;  accum csn = sum(mneg) = cs_half - 512
            mneg = wp.tile([128, NH], F16, tag="mneg")
            csn = wp.tile([128, 1], F32, tag="csn")
            nc.vector.scalar_tensor_tensor(mneg[:], h[:], -2.0, h[:],
                                           ALU.mult, ALU.mult, accum_out=csn[:])
            # fold the two n-halves on the PE: cs2[p'] = csn[p'&63] + csn[64+(p'&63)]
            cns16 = wp.tile([128, 2], F16, tag="cns16")
            nc.vector.tensor_copy(cns16[:, 0:1], csn[:])
            nc.vector.tensor_copy(cns16[:, 1:2], sscn[:])
            cs2 = ps_u.tile([128, 2], F32, tag="cs2")
            nc.tensor.matmul(cs2[:], fold[:], cns16[:], start=True, stop=True)
            cmat = wp.tile([128, NH], F16, tag="cmat")
            nc.vector.tensor_scalar(cmat[:], mneg[:], 1.0, None, ALU.add)

            # cos-side stationaries first; their matmuls overlap the sin side
            a_h0 = wp.tile([128, 4], F16, tag="a_h0")
            nc.vector.scalar_tensor_tensor(a_h0[:], gHH[:, 0:4], cs2[:, 0:1],
                                           gHH[:, 8:12], ALU.mult, ALU.add)
            a_h1 = wp.tile([128, 4], F16, tag="a_h1")
            nc.vector.scalar_tensor_tensor(a_h1[:], gHH[:, 4:8], cs2[:, 0:1],
                                           gHH[:, 12:16], ALU.mult, ALU.add)
            b_h0 = wp.tile([128, 4], F16, tag="b_h0")
            nc.vector.tensor_scalar(b_h0[:], gHH[:, 0:4], cs2[:, 1:2], None, ALU.mult)
            b_h1 = wp.tile([128, 4], F16, tag="b_h1")
            nc.vector.tensor_scalar(b_h1[:], gHH[:, 4:8], cs2[:, 1:2], None, ALU.mult)

            # e[(b c), n] per n-half; bank nh = a_hnh^T cmat + b_hnh^T smat
            e = ps_e.tile([4, N], F32, tag="e")
            nc.tensor.matmul(e[:, 0:512], a_h0[:], cmat[:], start=True, stop=False)
            nc.tensor.matmul(e[:, 512:1024], a_h1[:], cmat[:], start=True, stop=False)
            nc.tensor.matmul(e[:, 0:512], b_h0[:], smat[:], start=False, stop=True)
            nc.tensor.matmul(e[:, 512:1024], b_h1[:], smat[:], start=False, stop=True)

            es = wp.tile([4, N], F32, tag="es")
            nc.scalar.activation(es[:, 0:256], e[:, 0:256], AF.Copy)
            nc.vector.tensor_copy(es[:, 256:512], e[:, 256:512])
            nc.scalar.activation(es[:, 512:768], e[:, 512:768], AF.Copy)
            nc.vector.tensor_copy(es[:, 768:1024], e[:, 768:1024])
            # out rows (b0c0, b0c1, b1c0, b1c1) -> out[b, c, n] contiguous
            dst = out_t.rearrange("b c n -> (b c) n")
            nc.gpsimd.dma_start(dst[:, 0:512], es[:, 0:512])
            nc.sync.dma_start(dst[:, 512:1024], es[:, 512:1024])
    return nc


def _make_in_maps(x, shift0, shift1, amp0, amp1):
    kst12, gHHa, fold = _host_tables(shift0.reshape(-1)[0], shift1.reshape(-1)[0],
                                     amp0.reshape(-1)[0], amp1.reshape(-1)[0])
    t = np.asarray(x, np.float64) / (2.0 * np.pi)
    t0, t1, t2 = _split3(t)
    in_maps = []
    for c in range(NCORES):
        b0, b1 = BPC * c, BPC * c + 1
        t3p = np.zeros((8, NH), np.float16)
        for r_nh in range(2):
            for r_b, bb in ((0, b0), (1, b1)):
                for i, tt in enumerate((t0, t1)):
                    t3p[4 * r_nh + 2 * r_b + i] = tt[bb, NH * r_nh : NH * (r_nh + 1)]
        kt3 = np.concatenate([kst12, t3p], axis=1)               # [12, 640]
        in_maps.append({"kt3": kt3, "gHH": gHHa, "fold": fold})
    return in_maps


def kernel(x, shift0, shift1, amp0, amp1):
    in_maps = _make_in_maps(x, shift0, shift1, amp0, amp1)
    nc = _build_program()
    nc.finalize()
    res = run_bass_kernel_spmd(nc, in_maps, list(range(NCORES)))
    # device emits [BPC, 2, N]; reference wants [B, N, 2]
    out = np.concatenate([res.results[c]["out"] for c in range(NCORES)], axis=0)
    return np.ascontiguousarray(out.transpose(0, 2, 1)).astype(np.float32)


# revision 38
# speedup vs baseline: 1.0439x; 1.0439x over previous
"""Trainium2 Bass kernel for nn_NUFFTLayerMultiChannelInitMixed.

Math: the reference's spread->FFT->filter->IFFT->energy pipeline is an exact
bilinear form in the analytic spectrum of the periodized Gaussians.  With the
M-aliased images dropped (their weight is exp(-tau*(k-M)^2) ~ 3e-5) the
energy reduces to a truncated cosine series in the K lowest modes:

  e_i[n] = sum_{k<K} g_ik * ( cs_k cos(k x_n) + ss_k sin(k x_n) ) - self_i
  cs_k   = sum_n cos(k x_n),   ss_k = sum_n sin(k x_n)
  g_ik   = pref * w_k * deconv^2_k * mult_ik * p_k^2   (host precomputed)

K=32 keeps rel err ~4e-4 (gate is 2e-2).  Each core handles BPC=2 batches;
partition p = 64*nhalf + 32*batch + k packs BOTH 512-point halves of BOTH
batches into the 128 partitions, so every elementwise stage is a single
[128, 512] instruction:

  phases   u = kst8^T @ t3p on PE (fp16 2-term split of t: phase err ~1e-4 rad)
  range    v = u + MAGIC; negr = (v - MAGIC) - u   (exact fp32 rounding trick)
  trig     h = sin(pi*negr); smat = sin(2*pi*u) via Sin ACT (accum -> ss)
           mneg = -2h^2 = cos - 1 (accum -> cs_half-512); cmat = mneg + 1
  energy   e = a^T cos + b^T sin, four block-masked fp16 stationaries
           (per n-half), self-energy and the +1024 fold baked host-side

No transposes, no collectives; output written [b, c, n] and transposed on
the host.
"""

import numpy as np

try:
    import concourse.bass as bass
except ImportError:
    import sys
    sys.path.insert(0, "/opt/trn_rl_repo")
    import concourse.bass as bass

import concourse.bacc as bacc
import concourse.mybir as mybir
from concourse import tile
from concourse.bass_utils import run_bass_kernel_spmd

F32 = mybir.dt.float32
F16 = mybir.dt.float16
AF = mybir.ActivationFunctionType
ALU = mybir.AluOpType

M = 2001
L = 2.0 * np.pi
TAU = 12.0 * (L / (2.0 * np.pi * M)) ** 2
K = 32                  # modes kept per batch
B_FULL, N = 16, 1024
NH = 512                # points per n-half
NCORES = 8
BPC = B_FULL // NCORES
MAGIC = 12582912.0      # 1.5 * 2^23: fl(u + MAGIC) - MAGIC = round-to-nearest(u)
PI = float(np.pi)


def _host_tables(shift0, shift1, amp0, amp1):
    """fp64 k-space tables for the packed layout p = 64*nh + 32*b + k.

    Returns kst12 [12,128] fp16 (phase stationary, rows r = 6*nh + 3*b + i)
    and gHHa [128,16] f32 = [gM_h0 | gM_h1 | HHa_h0 | HHa_h1]."""
    k = np.arange(K, dtype=np.float64)
    p = np.exp(-TAU * k * k)
    Cc = (M / L) * np.sqrt(4.0 * np.pi * TAU)
    deconv2 = (np.pi / TAU) * np.exp(2.0 * TAU * k * k)
    mult1 = float(amp0) * 4.0 * np.pi / (k * k + (1.0 * float(shift0)) ** 2)
    mult2 = float(amp1) * 4.0 * np.pi / (k * k + (0.5 * float(shift1)) ** 2)
    w = np.full(K, 2.0)
    w[0] = 1.0
    scale = 1.0 / ((2.0 * np.pi * M / L) * (2.0 * np.pi))
    pref = scale * Cc * Cc / M
    g = np.stack([pref * w * deconv2 * mult1 * p * p,
                  pref * w * deconv2 * mult2 * p * p], axis=1)   # [K, 2]
    self2 = g.sum(axis=0)

    pp = np.arange(128)
    nh_p, b_p, k_p = pp // 64, (pp // 32) % 2, pp % 32
    gM = np.zeros((2, 128, 4))
    HHa = np.zeros((2, 128, 4))
    for nh in range(2):
        rows = np.nonzero(nh_p == nh)[0]
        for r in rows:
            for c in range(2):
                col = 2 * b_p[r] + c
                gM[nh, r, col] = g[k_p[r], c]
                # cs arrives as csneg = cs - 1024: fold 1024*g into the bias
                HHa[nh, r, col] = 1024.0 * g[k_p[r], c] - (
                    self2[c] if k_p[r] == 0 else 0.0)
    gHHa = np.concatenate([gM[0], gM[1], HHa[0], HHa[1]],
                          axis=1).astype(np.float32)             # [128, 16]

    kst12 = np.zeros((8, 128), np.float32)
    for r in range(8):
        r_nh, r_b = r // 4, (r // 2) % 2
        sel = (nh_p == r_nh) & (b_p == r_b)
        kst12[r, sel] = k_p[sel]

    # fold matrix: cs2[p'] = sum_p Fold[p, p'] csh[p] adds the two n-halves
    fold = (pp[:, None] % 64 == pp[None, :] % 64).astype(np.float16)
    return kst12.astype(np.float16), gHHa, fold


def _split3(t):
    """t fp64 -> three fp16 arrays with t0+t1+t2 == t to ~2^-25."""
    t0 = t.astype(np.float16)
    r = t - t0.astype(np.float64)
    t1 = r.astype(np.float16)
    r = r - t1.astype(np.float64)
    t2 = r.astype(np.float16)
    return t0, t1, t2


def _build_program(debug=False):
    nc = bacc.Bacc(None, target_bir_lowering=False, debug=debug)
    # kt3 = [kst12 | t3p] packed fp16; gHHa = [gM_h0|gM_h1|HHa_h0|HHa_h1] f32
    kt3_in = nc.declare_dram_parameter("kt3", [8, 128 + NH], F16, isOutput=False)
    gHH_in = nc.declare_dram_parameter("gHH", [128, 16], F32, isOutput=False)
    fold_in = nc.declare_dram_parameter("fold", [128, 128], F16, isOutput=False)
    out_t = nc.declare_dram_parameter("out", [BPC, 2, N], F32, isOutput=True)

    with tile.TileContext(nc) as tc:
        import contextlib
        with contextlib.ExitStack() as ctx:
            pc = ctx.enter_context(tc.tile_pool(name="const", bufs=1))
            wp = ctx.enter_context(tc.tile_pool(name="work", bufs=1))
            ps_u = ctx.enter_context(tc.tile_pool(name="psu", bufs=1, space="PSUM"))
            ps_e = ctx.enter_context(tc.tile_pool(name="pse", bufs=1, space="PSUM"))

            kt3 = pc.tile([8, 128 + NH], F16, tag="kt3")
            nc.sync.dma_start(kt3[:], kt3_in[:])
            gHH = pc.tile([128, 16], F32, tag="gHH")
            nc.scalar.dma_start(gHH[:], gHH_in[:])
            fold = pc.tile([128, 128], F16, tag="fold")
            nc.sync.dma_start(fold[:], fold_in[:])
            kst = kt3[:, 0:128]
            t3p = kt3[:, 128 : 128 + NH]

            # phases u[p, n'] = k(p) * t_{b(p)}[512*nh(p) + n']
            u = ps_u.tile([128, NH], F32, tag="u")
            nc.tensor.matmul(u[:], kst, t3p, start=True, stop=True)

            # negr = round(u) - u (exact); sin(-2*pi*negr) = sin(2*pi*u)
            v = wp.tile([128, NH], F32, tag="v")
            nc.vector.tensor_scalar(v[:], u[:], MAGIC, None, ALU.add)
            negr = wp.tile([128, NH], F32, tag="negr")
            nc.vector.scalar_tensor_tensor(negr[:], v[:], MAGIC, u[:],
                                           ALU.subtract, ALU.subtract)

            h = wp.tile([128, NH], F16, tag="h")
            nc.scalar.activation(h[:], negr[:], AF.Sin, scale=PI)
            sscn = wp.tile([128, 1], F32, tag="sscn")
            smat = wp.tile([128, NH], F16, tag="smat")
            nc.scalar.activation(smat[:], negr[:], AF.Sin, scale=-2.0 * PI,
                                 accum_out=sscn[:])

            # mneg = -2 sin^2(pi r) = cos - 1;  accum csn = sum(mneg) = cs_half - 512
            mneg = wp.tile([128, NH], F16, tag="mneg")
            csn = wp.tile([128, 1], F32, tag="csn")
            nc.vector.scalar_tensor_tensor(mneg[:], h[:], -2.0, h[:],
                                           ALU.mult, ALU.mult, accum_out=csn[:])
            # fold the two n-halves on the PE: cs2[p'] = csn[p'&63] + csn[64+(p'&63)]
            cns16 = wp.tile([128, 2], F16, tag="cns16")
            nc.vector.tensor_copy(cns16[:, 0:1], csn[:])
            nc.vector.tensor_copy(cns16[:, 1:2], sscn[:])
            cs2 = ps_u.tile([128, 2], F32, tag="cs2")
            nc.tensor.matmul(cs2[:], fold[:], cns16[:], start=True, stop=True)
            cmat = wp.tile([128, NH], F16, tag="cmat")
            nc.vector.tensor_scalar(cmat[:], mneg[:], 1.0, None, ALU.add)

            # cos-side stationaries first; their matmuls overlap the sin side
            a_h0 = wp.tile([128, 4], F16, tag="a_h0")
            nc.vector.scalar_tensor_tensor(a_h0[:], gHH[:, 0:4], cs2[:, 0:1],
                                           gHH[:, 8:12], ALU.mult, ALU.add)
            a_h1 = wp.tile([128, 4], F16, tag="a_h1")
            nc.vector.scalar_tensor_tensor(a_h1[:], gHH[:, 4:8], cs2[:, 0:1],
                                           gHH[:, 12:16], ALU.mult, ALU.add)
            b_h0 = wp.tile([128, 4], F16, tag="b_h0")
            nc.vector.tensor_scalar(b_h0[:], gHH[:, 0:4], cs2[:, 1:2], None, ALU.mult)
            b_h1 = wp.tile([128, 4], F16, tag="b_h1")
            nc.vector.tensor_scalar(b_h1[:], gHH[:, 4:8], cs2[:, 1:2], None, ALU.mult)

            # e[(b c), n] per n-half; bank nh = a_hnh^T cmat + b_hnh^T smat
            e = ps_e.tile([4, N], F32, tag="e")
            nc.tensor.matmul(e[:, 0:512], a_h0[:], cmat[:], start=True, stop=False)
            nc.tensor.matmul(e[:, 512:1024], a_h1[:], cmat[:], start=True, stop=False)
            nc.tensor.matmul(e[:, 0:512], b_h0[:], smat[:], start=False, stop=True)
            nc.tensor.matmul(e[:, 512:1024], b_h1[:], smat[:], start=False, stop=True)

            es = wp.tile([4, N], F32, tag="es")
            nc.scalar.activation(es[:, 0:512], e[:, 0:512], AF.Copy)
            nc.vector.tensor_copy(es[:, 512:1024], e[:, 512:1024])
            # out rows (b0c0, b0c1, b1c0, b1c1) -> out[b, c, n] contiguous
            dst = out_t.rearrange("b c n -> (b c) n")
            nc.gpsimd.dma_start(dst[:, 0:512], es[:, 0:512])
            nc.sync.dma_start(dst[:, 512:1024], es[:, 512:1024])
    return nc


def _make_in_maps(x, shift0, shift1, amp0, amp1):
    kst12, gHHa, fold = _host_tables(shift0.reshape(-1)[0], shift1.reshape(-1)[0],
                                     amp0.reshape(-1)[0], amp1.reshape(-1)[0])
    t = np.asarray(x, np.float64) / (2.0 * np.pi)
    t0, t1, t2 = _split3(t)
    in_maps = []
    for c in range(NCORES):
        b0, b1 = BPC * c, BPC * c + 1
        t3p = np.zeros((8, NH), np.float16)
        for r_nh in range(2):
            for r_b, bb in ((0, b0), (1, b1)):
                for i, tt in enumerate((t0, t1)):
                    t3p[4 * r_nh + 2 * r_b + i] = tt[bb, NH * r_nh : NH * (r_nh + 1)]
        kt3 = np.concatenate([kst12, t3p], axis=1)               # [12, 640]
        in_maps.append({"kt3": kt3, "gHH": gHHa, "fold": fold})
    return in_maps


def kernel(x, shift0, shift1, amp0, amp1):
    in_maps = _make_in_maps(x, shift0, shift1, amp0, amp1)
    nc = _build_program()
    nc.finalize()
    res = run_bass_kernel_spmd(nc, in_maps, list(range(NCORES)))
    # device emits [BPC, 2, N]; reference wants [B, N, 2]
    out = np.concatenate([res.results[c]["out"] for c in range(NCORES)], axis=0)
    return np.ascontiguousarray(out.transpose(0, 2, 1)).astype(np.float32)
